# revision 35
# baseline (speedup 1.0000x reference)
"""Trainium2 Bass kernel v3 for nn_Capsule_2731599200537 (capsule routing).

Math (per core, i-sharded NIS=256):
    votes[b,i,ja] = sum_k x[b,i,k] W[i,k,ja]
    r1: preact = (sum_i votes)/33        (dense (ik) matmul, no votes needed)
    r>1: logits_r = votes . actsum_{r-1} (actsum = act1+...+act_{r-1}; linearity)
         route = leaky-softmax(logits); preact = route . votes via xr=x*route
    AllReduce preact partials each round; act = squash(preact + b).

v3 changes vs v2:
  - All input layouts (xb block-diag, w16, w2, xt16, xt2k) are prepared on
    the HOST in fp16 and DMA'd directly -> no on-device converts/transposes
    and no mask/tensor_tensor xb production.
  - Squash uses sqrt(x) = exp(0.5*ln(x)) so every activation comes from the
    one `natural_log_exp_and_others` table -> no act-table reloads.
  - Votes eviction merged into [128,1024] copies, alternating DVE/ACT.
  - Collective partials in fp16; cc mode selectable (ar32/ar16/ag16).
"""
from contextlib import nullcontext

import numpy as np

import concourse.bacc as bacc
import concourse.mybir as mybir
from concourse import tile
from concourse.masks import make_identity

B = 64
NI = 2048
KA = 8
NO = 32
AT = 16
O = NO * AT
R = 3
NCORES = 8
NIS = NI // NCORES
NH = NIS // 128
NT = NIS // 16

F32 = mybir.dt.float32
F16 = mybir.dt.float16
AF = mybir.ActivationFunctionType
AX = mybir.AxisListType
MUL = mybir.AluOpType.mult


def build(n_cores: int = NCORES, use_collective: bool = True,
          cc_mode: str = "ag16", loop_n: int | None = None, parts: int = 4,
          cc_chunks: int = 2):
    nc = bacc.Bacc(None, target_bir_lowering=False, debug=False,
                   num_devices=n_cores)
    # host-prepped fp16 inputs
    xb_d = nc.dram_tensor("xb", [NT, 128, B * 16], F16, kind="ExternalInput")
    w16_d = nc.dram_tensor("w16", [128, NT, O], F16, kind="ExternalInput")
    w2_d = nc.dram_tensor("w2", [NH, 128, KA * O], F16, kind="ExternalInput")
    xt16_d = nc.dram_tensor("xt16", [128, NT, B], F16, kind="ExternalInput")
    xt2k_d = nc.dram_tensor("xt2k", [NH, 128, KA * B], F16,
                            kind="ExternalInput")
    b_d = nc.dram_tensor("b", [NO, AT], F32, kind="ExternalInput")
    y_d = nc.dram_tensor("y", [B, NO, AT], F32, kind="ExternalOutput")

    with tile.TileContext(nc) as tc:
        with (
            tc.tile_pool(name="big", bufs=1) as big,
            tc.tile_pool(name="cst", bufs=1) as cst,
            tc.tile_pool(name="psA", bufs=3, space="PSUM") as psA,
            tc.tile_pool(name="psT", bufs=1, space="PSUM") as psT,
            tc.tile_pool(name="psB", bufs=1, space="PSUM") as psB,
            tc.tile_pool(name="dram", bufs=2, space="DRAM") as dram,
        ):
            # ---- persistent SBUF ----
            v8 = [big.tile([128, B * NIS], F16, tag=f"v8_{c}", name=f"v8_{c}")
                  for c in range(4)]
            v8v = [t.rearrange("p (b i) -> p b i", b=B) for t in v8]
            w16 = big.tile([128, NT * O], F16, tag="w16", name="w16")
            w16v = w16.rearrange("p (t o) -> p t o", t=NT)
            w2 = [big.tile([128, KA * O], F16, tag=f"w2_{h}", name=f"w2_{h}")
                  for h in range(NH)]
            w2v = [t.rearrange("p (k o) -> p k o", k=KA) for t in w2]
            xt16 = big.tile([128, NT * B], F16, tag="xt16", name="xt16")
            xt16v = xt16.rearrange("p (t b) -> p t b", t=NT)
            xt2k = [big.tile([128, KA * B], F16, tag=f"xt2k{h}", name=f"x2k{h}")
                    for h in range(NH)]
            xt2kv = [t.rearrange("p (k b) -> p k b", k=KA) for t in xt2k]

            ident = cst.tile([64, 64], F16)
            make_identity(nc, ident[:])
            bias_bc = cst.tile([64, O], F32)

            # 4-slot PSUM transpose buffer: transposes don't serialize
            ptq = psT.tile([128, 256], F16, tag="ptT", name="ptq")
            slot = [0]

            def transpose_evict(src_ap, dst_ap, eng, p0=0):
                s = slot[0] % 4
                slot[0] += 1
                n = src_ap.shape[0]
                nc.tensor.transpose(ptq[:, 64 * s:64 * s + n], src_ap,
                                    ident[p0:p0 + n, p0:p0 + n])
                if eng is nc.scalar:
                    nc.scalar.copy(dst_ap, ptq[:, 64 * s:64 * s + n])
                else:
                    eng.tensor_copy(dst_ap, ptq[:, 64 * s:64 * s + n])

            _loop = tc.For_i(0, loop_n, 1) if loop_n else nullcontext()
            _loop.__enter__()
            # =========== phase 1: loads, r1 preact, votes ========
            with tc.tile_pool(name="xbp", bufs=4) as xbp:
                brow = xbp.tile([1, O], F32, tag="brow")
                nc.sync.dma_start(
                    brow[:], b_d.rearrange("j a -> (j a)").unsqueeze(0))
                nc.gpsimd.partition_broadcast(bias_bc[:], brow[:])
                # maskJ for round actblk production [128 (ja), 8 (j-of-chunk)]
                maskJ = cst.tile([128, 8], F16, tag="maskJ")
                nc.gpsimd.memset(maskJ[:], 1.0)
                nc.gpsimd.affine_select(
                    out=maskJ[:], in_=maskJ[:],
                    compare_op=mybir.AluOpType.is_ge, fill=0.0,
                    base=0, pattern=[[-16, 8]], channel_multiplier=1)
                nc.gpsimd.affine_select(
                    out=maskJ[:], in_=maskJ[:],
                    compare_op=mybir.AluOpType.is_ge, fill=0.0,
                    base=15, pattern=[[16, 8]], channel_multiplier=-1)

                if parts >= 1:
                    nc.sync.dma_start(xt16[:],
                                      xt16_d.rearrange("p t b -> p (t b)"))

                # ---- pipelined: xb/w16 DMA -> r1-preact chunk -> votes ----
                psb1 = psB.tile([64, O], F32, tag="psb")
                for t in range(NT if parts >= 2 else 0):
                    if t % 2 == 0:
                        nc.sync.dma_start(
                            w16v[:, t:t + 2, :], w16_d[:, t:t + 2, :])
                    xbt = xbp.tile([128, B * 16], F16, tag="xb")
                    nc.sync.dma_start(xbt[:], xb_d[t])
                    nc.tensor.matmul(psb1[:], xt16v[:, t, :], w16v[:, t, :],
                                     start=(t == 0), stop=(t == NT - 1))
                    for c2 in range(4):
                        bank = psA.tile([128, 1024], F32, tag="bank")
                        for h in range(2):
                            nc.tensor.matmul(
                                bank[:, 512 * h:512 * (h + 1)],
                                w16v[:, t, 128 * c2:128 * (c2 + 1)],
                                xbt[:, 512 * h:512 * (h + 1)],
                                start=True, stop=True)
                        bkv = bank.rearrange("p (b i) -> p b i", b=B)
                        idx = (t * 4 + c2) % 16
                        if idx % 2 == 0:
                            nc.vector.tensor_copy(
                                v8v[c2][:, :, 16 * t:16 * (t + 1)], bkv)
                        else:
                            nc.scalar.copy(
                                v8v[c2][:, :, 16 * t:16 * (t + 1)], bkv)
                # deferred loads (needed only from round-2 preact onward)
                if parts >= 1:
                    for h in range(NH):
                        nc.sync.dma_start(xt2k[h][:], xt2k_d[h])
                        for c in range(2):
                            nc.sync.dma_start(
                                w2[h][:, 2048 * c:2048 * (c + 1)],
                                w2_d[h][:, 2048 * c:2048 * (c + 1)])

            # =========== routing ===========
            with (
                tc.tile_pool(name="rt", bufs=1) as rt,
                tc.tile_pool(name="xrp", bufs=2) as xrp,
            ):
                routef = [rt.tile([128, NO * B], F16, tag=f"rf{h}", name=f"rf{h}")
                          for h in range(NH)]          # [i, (j, b)]
                rfv = [t.rearrange("p (j b) -> p j b", j=NO) for t in routef]
                actT = [rt.tile([128, 64], F16, tag=f"actT{c}", name=f"acT{c}")
                        for c in range(4)]
                av = [rt.tile([128, B * 8], F16, tag=f"ab{c}", name=f"ab{c}")
                      for c in range(4)]               # f16 actblk
                avv = [t.rearrange("p (b j) -> p b j", b=B) for t in av]

                xd = [rt.tile([128, KA * B], F16, tag=f"xd{h}", name=f"xd{h}")
                      for h in range(NH)]
                xdv = [t.rearrange("p (k b) -> p k b", k=KA) for t in xd]
                pre_part = rt.tile([64, O], F32, tag="pre_part")
                pre_sum = rt.tile([64, O], F32, tag="pre_sum")
                sq = pre_part  # pre_part is dead once pre_sum exists
                nsq = rt.tile([64, NO], F32, tag="nsq")
                norm = rt.tile([64, NO], F32, tag="norm")
                d1 = rt.tile([64, NO], F32, tag="d1")
                rd = rt.tile([64, NO], F32, tag="rd")
                fs = rt.tile([64, NO], F32, tag="fs")
                acts = rt.tile([64, O], F32, tag="acts")
                act16 = rt.tile([64, O], F16, tag="act16")
                asum32 = rt.tile([64, O], F32, tag="asum32")
                asum16 = rt.tile([64, O], F16, tag="asum16")
                denom = [rt.tile([128, B], F32, tag=f"den{h}", name=f"den{h}")
                         for h in range(NH)]
                recip = [rt.tile([128, B], F32, tag=f"rec{h}", name=f"rec{h}")
                         for h in range(NH)]
                recip16 = [rt.tile([128, B], F16, tag=f"re6{h}", name=f"re6{h}")
                           for h in range(NH)]
                ccbuf16 = rt.tile([64, 8 * (O // cc_chunks)], F16, tag="ccg")
                ccv = ccbuf16.rearrange("p (c o) -> p c o", c=8)
                pp16 = rt.tile([64, O], F16, tag="pp16")

                def cc_reduce(r):
                    """pre_part -> pre_sum (+bias), chunked collective."""
                    nch = cc_chunks
                    W_ = O // nch
                    for ch in range(nch):
                        sl = slice(W_ * ch, W_ * (ch + 1))
                        if use_collective and cc_mode != "none":
                            if cc_mode == "ar32":
                                cc_in = dram.tile([64, W_], F32, tag="ccin")
                                cc_out = dram.tile([64, W_], F32, tag="ccout")
                                nc.sync.dma_start(cc_in[:], pre_part[:, sl])
                                nc.gpsimd.collective_compute(
                                    "AllReduce", mybir.AluOpType.add,
                                    replica_groups=[list(range(n_cores))],
                                    ins=[cc_in.opt()], outs=[cc_out.opt()])
                                nc.sync.dma_start(pre_sum[:, sl], cc_out[:])
                                nc.vector.tensor_add(pre_sum[:, sl],
                                                     pre_sum[:, sl],
                                                     bias_bc[:, sl])
                            elif cc_mode == "ar16":
                                cc_in = dram.tile([64, W_], F16, tag="ccin")
                                cc_out = dram.tile([64, W_], F16, tag="ccout")
                                nc.scalar.copy(pp16[:, sl], pre_part[:, sl])
                                nc.sync.dma_start(cc_in[:], pp16[:, sl])
                                nc.gpsimd.collective_compute(
                                    "AllReduce", mybir.AluOpType.add,
                                    replica_groups=[list(range(n_cores))],
                                    ins=[cc_in.opt()], outs=[cc_out.opt()])
                                nc.sync.dma_start(pp16[:, sl], cc_out[:])
                                nc.vector.tensor_tensor(
                                    pre_sum[:, sl], pp16[:, sl],
                                    bias_bc[:, sl],
                                    op=mybir.AluOpType.add)
                            else:  # ag16: AllGather + local reduce
                                cc_in = dram.tile([64, W_], F16, tag="ccin")
                                cc_out = dram.tile([64 * 8, W_], F16,
                                                   tag="ccout")
                                nc.gpsimd.tensor_copy(pp16[:, sl],
                                                      pre_part[:, sl])
                                nc.sync.dma_start(cc_in[:], pp16[:, sl])
                                nc.gpsimd.collective_compute(
                                    "AllGather", mybir.AluOpType.bypass,
                                    replica_groups=[list(range(n_cores))],
                                    ins=[cc_in.opt()], outs=[cc_out.opt()])
                                gv = ccv
                                nc.sync.dma_start(
                                    gv,
                                    cc_out.rearrange("(c p) o -> p c o", c=8))
                                # single strided reduce over the 8 replicas
                                nc.vector.reduce_sum(
                                    pre_sum[:, sl],
                                    gv.transpose([0, 2, 1]), axis=AX.X)
                                nc.vector.tensor_add(pre_sum[:, sl],
                                                     pre_sum[:, sl],
                                                     bias_bc[:, sl])
                        else:
                            nc.scalar.copy(pre_sum[:, sl], pre_part[:, sl])
                            nc.vector.tensor_add(pre_sum[:, sl],
                                                 pre_sum[:, sl],
                                                 bias_bc[:, sl])

                def squash(r):
                    """pre_sum -> acts -> act16/asum; per cc-chunk slices."""
                    nch = cc_chunks
                    JW = NO // nch
                    for ch in range(nch):
                        sl = slice(AT * JW * ch, AT * JW * (ch + 1))
                        jl = slice(JW * ch, JW * (ch + 1))
                        nc.vector.tensor_mul(sq[:, sl], pre_sum[:, sl],
                                             pre_sum[:, sl])
                        nc.vector.reduce_sum(
                            nsq[:, jl],
                            sq[:, sl].rearrange("p (j a) -> p j a", a=AT),
                            axis=AX.X)
                        nc.scalar.activation(norm[:, jl], nsq[:, jl], AF.Sqrt)
                        nc.vector.tensor_scalar_add(d1[:, jl], nsq[:, jl], 1.0)
                        nc.vector.reciprocal(rd[:, jl], d1[:, jl])
                        nc.vector.tensor_mul(fs[:, jl], norm[:, jl], rd[:, jl])
                        nc.vector.tensor_mul(
                            acts[:, sl].rearrange("p (j a) -> p j a", a=AT),
                            pre_sum[:, sl].rearrange("p (j a) -> p j a", a=AT),
                            fs[:, jl].unsqueeze(2).broadcast_to([64, JW, AT]))
                        if r < R:
                            nc.gpsimd.tensor_copy(act16[:, sl], acts[:, sl])
                            if r == 1:
                                nc.gpsimd.tensor_copy(asum32[:, sl],
                                                      acts[:, sl])
                            else:
                                nc.gpsimd.tensor_tensor(
                                    asum32[:, sl], asum32[:, sl],
                                    acts[:, sl], op=mybir.AluOpType.add)
                                nc.gpsimd.tensor_copy(asum16[:, sl],
                                                      asum32[:, sl])
                        else:
                            nc.sync.dma_start(
                                y_d.rearrange("b j a -> b (j a)")[:, sl],
                                acts[:, sl])

                # ---- round 1 ----
                # high priority: this short chain gates round 2 and must
                # jump ahead of the votes-eviction backlog in engine queues
                if parts >= 2:
                    with tc.high_priority():
                        nc.scalar.mul(pre_part[:], psb1[:], 1.0 / 33.0)
                        cc_reduce(1)
                        squash(1)

                # ---- rounds 2..R ----
                for r in range(2, (R + 1) if parts >= 3 else 2):
                    actsrc = act16 if r == 2 else asum16
                    # actT + actblk, split by b-half so round-r work can
                    # start as soon as the first squash chunk lands
                    with tc.high_priority():
                        for c in range(4):
                            for bh in range(2):
                                bsl = slice(32 * bh, 32 * (bh + 1))
                                transpose_evict(
                                    actsrc[bsl, 128 * c:128 * (c + 1)],
                                    actT[c][:, bsl], nc.scalar, p0=32 * bh)
                                nc.gpsimd.tensor_tensor(
                                    avv[c][:, bsl, :],
                                    maskJ.unsqueeze(1)
                                    .broadcast_to([128, 32, 8]),
                                    actT[c][:, bsl].unsqueeze(2)
                                    .broadcast_to([128, 32, 8]),
                                    op=MUL)

                    # dlogit -> exp -> denom -> xd, pipelined per (h, bhalf)
                    for h in range(NH):
                        for bh in range(2):
                            bank = psA.tile([128, 1024], F32, tag="bank")
                            for bgs in range(2):
                                bg = 2 * bh + bgs
                                for b16 in range(16):
                                    b = 16 * bg + b16
                                    for c in range(4):
                                        off = 512 * bgs + 32 * b16 + 8 * c
                                        nc.tensor.matmul(
                                            bank[:, off:off + 8],
                                            v8v[c][:, b,
                                                   128 * h:128 * (h + 1)],
                                            avv[c][:, b, :],
                                            start=(c == 0), stop=(c == 3))
                            # exp: bank [i,(b32,j32)] -> rfv [i,(j,b32 slice)]
                            bkv = bank.rearrange("p (b j) -> p b j", b=32)
                            bsl = slice(32 * bh, 32 * (bh + 1))
                            with tc.high_priority():
                                nc.scalar.activation(
                                    rfv[h][:, :, bsl].transpose([0, 2, 1]),
                                    bkv, AF.Exp)
                                nc.vector.reduce_sum(
                                    denom[h][:, bsl],
                                    rfv[h][:, :, bsl].transpose([0, 2, 1]),
                                    axis=AX.X)
                                nc.vector.tensor_scalar_add(
                                    denom[h][:, bsl], denom[h][:, bsl], 1.0)
                                nc.vector.reciprocal(recip[h][:, bsl],
                                                     denom[h][:, bsl])
                                nc.gpsimd.tensor_copy(recip16[h][:, bsl],
                                                      recip[h][:, bsl])
                                nc.vector.tensor_mul(
                                    xdv[h][:, :, bsl],
                                    xt2kv[h][:, :, bsl],
                                    recip16[h][:, bsl].unsqueeze(1)
                                    .broadcast_to([128, KA, 32]))

                    # xr -> preact matmuls, streamed per j-group
                    psb = psB.tile([64, O], F32, tag="psb")
                    for jg in range(8):
                        xrt = [xrp.tile([128, 4 * KA * B], F16, tag="xr",
                                        name=f"xrt{h}")
                               for h in range(NH)]
                        for h in range(NH):
                            xv = xrt[h].rearrange("p (j k b) -> p j k b",
                                                  j=4, k=KA)
                            eng = nc.vector
                            for bh in range(2):
                                bsl = slice(32 * bh, 32 * (bh + 1))
                                eng.tensor_tensor(
                                    xv[:, :, :, bsl],
                                    xdv[h][:, :, bsl].unsqueeze(1)
                                    .broadcast_to([128, 4, KA, 32]),
                                    rfv[h][:, 4 * jg:4 * (jg + 1), bsl]
                                    .unsqueeze(2)
                                    .broadcast_to([128, 4, KA, 32]),
                                    op=MUL)
                        for jj in range(4):
                            j = 4 * jg + jj
                            for h in range(NH):
                                for k in range(KA):
                                    nc.tensor.matmul(
                                        psb[:, 16 * j:16 * (j + 1)],
                                        xrt[h][:, 512 * jj + 64 * k:
                                               512 * jj + 64 * (k + 1)],
                                        w2v[h][:, k, 16 * j:16 * (j + 1)],
                                        start=(h == 0 and k == 0),
                                        stop=(h == NH - 1 and k == KA - 1))
                        # evict finished j-slices promptly so chunked cc
                        # can overlap the remaining j-groups
                        if jg == 3:
                            with tc.high_priority():
                                nc.scalar.copy(pre_part[:, :256],
                                               psb[:, :256])
                        elif jg == 7:
                            with tc.high_priority():
                                nc.scalar.copy(pre_part[:, 256:],
                                               psb[:, 256:])
                    with tc.high_priority():
                        cc_reduce(r)
                        squash(r)
            _loop.__exit__(None, None, None)

    nc.compile()
    return nc


_NC_CACHE = {}


def _get_nc(n_cores=NCORES, use_collective=True, cc_mode="ag16"):
    key = (n_cores, use_collective, cc_mode)
    if key not in _NC_CACHE:
        _NC_CACHE[key] = build(n_cores, use_collective, cc_mode=cc_mode)
    return _NC_CACHE[key]


class Runner:
    """Compiles the Bass module to a PJRT executable once; reusable calls."""

    def __init__(self, nc, n_cores=NCORES):
        import jax
        import concourse.mybir as _mybir
        from concourse import bass2jax as b2j
        from jax.experimental.shard_map import shard_map
        from jax.sharding import Mesh, PartitionSpec

        b2j.install_neuronx_cc_hook()
        self.nc = nc
        self.n_cores = n_cores
        pname = nc.partition_id_tensor.name if nc.partition_id_tensor else None
        in_names, out_names, out_avals, zero_outs = [], [], [], []
        for alloc in nc.m.functions[0].allocations:
            if not isinstance(alloc, _mybir.MemoryLocationSet):
                continue
            name = alloc.memorylocations[0].name
            if alloc.kind == "ExternalInput":
                if name != pname:
                    in_names.append(name)
            elif alloc.kind == "ExternalOutput":
                shape = tuple(alloc.tensor_shape)
                dtype = _mybir.dt.np(alloc.dtype)
                out_names.append(name)
                out_avals.append(jax.core.ShapedArray(shape, dtype))
                zero_outs.append(np.zeros(shape, dtype))
        self.in_names = list(in_names)
        self.out_names = out_names
        self.out_avals = out_avals
        self.zero_outs = zero_outs
        n_params = len(in_names)
        all_names = in_names + out_names + ([pname] if pname else [])
        donate = tuple(range(n_params, n_params + len(out_names)))
        self.n_params = n_params

        def _body(*args):
            operands = list(args)
            if pname is not None:
                operands.append(b2j.partition_id_tensor())
            outs = b2j._bass_exec_p.bind(
                *operands,
                out_avals=tuple(out_avals),
                in_names=tuple(all_names),
                out_names=tuple(out_names),
                lowering_input_output_aliases=(),
                sim_require_finite=False,
                sim_require_nnan=False,
                nc=nc,
            )
            return tuple(outs)

        devices = jax.devices()[:n_cores]
        mesh = Mesh(np.asarray(devices), ("core",))
        nio = n_params + len(out_names)
        self._jit = jax.jit(
            shard_map(_body, mesh=mesh,
                      in_specs=(PartitionSpec("core"),) * nio,
                      out_specs=(PartitionSpec("core"),) * len(out_names),
                      check_rep=False),
            donate_argnums=donate, keep_unused=True)

    def __call__(self, in_maps, block=True):
        n = self.n_cores
        concat_in = [
            np.concatenate([np.asarray(in_maps[c][name]) for c in range(n)],
                           axis=0)
            for name in self.in_names
        ]
        concat_zero = [
            np.zeros((n * z.shape[0], *z.shape[1:]), z.dtype)
            for z in self.zero_outs
        ]
        out = self._jit(*concat_in, *concat_zero)
        if block:
            for o in out:
                o.block_until_ready()
        return [
            {name: np.asarray(out[i]).reshape(n, *self.out_avals[i].shape)[c]
             for i, name in enumerate(self.out_names)}
            for c in range(n)
        ]


_RUNNER_CACHE = {}


def get_runner(n_cores=NCORES, use_collective=True, cc_mode="ag16"):
    key = (n_cores, use_collective, cc_mode)
    if key not in _RUNNER_CACHE:
        _RUNNER_CACHE[key] = Runner(
            _get_nc(n_cores, use_collective, cc_mode), n_cores)
    return _RUNNER_CACHE[key]


def make_in_maps(x, W, b, n_cores=NCORES):
    """Host-side sharding + fp16 layout preparation (per core)."""
    x = np.asarray(x, dtype=np.float32)
    W = np.asarray(W, dtype=np.float32)
    b = np.asarray(b, dtype=np.float32)
    x16 = x.astype(np.float16)
    W16 = W.astype(np.float16)
    maps = []
    for c in range(n_cores):
        sl = slice(c * NIS, (c + 1) * NIS)
        xc = x16[:, sl, :]                    # [B, NIS, KA]
        Wc = W16[sl]                          # [NIS, KA, O]
        # w16 [128 (i16 k8), NT, O]: p = 8*i1 + k, i = 16*t + i1
        w16 = np.ascontiguousarray(
            Wc.reshape(NT, 16, KA, O).transpose(1, 2, 0, 3).reshape(128, NT, O))
        # w2 [NH, 128 i, KA*O]
        w2 = np.ascontiguousarray(
            Wc.reshape(NH, 128, KA * O))
        # xt16 [128 (i16 k8), NT, B]: value x[b, 16t+i1, k]
        xt = xc.transpose(1, 2, 0)            # [NIS, KA, B]
        xt16 = np.ascontiguousarray(
            xt.reshape(NT, 16, KA, B).transpose(1, 2, 0, 3).reshape(128, NT, B))
        # xt2k [NH, 128 i, KA, B] -> [NH, 128, KA*B]
        xt2k = np.ascontiguousarray(
            xt.reshape(NH, 128, KA, B).reshape(NH, 128, KA * B))
        # xb [NT, 128, B, 16] block-diag: xb[t, 8i1+k, b, i1] = x[b,16t+i1,k]
        xb = np.zeros((NT, 16, KA, B, 16), dtype=np.float16)
        xsrc = xc.reshape(B, NT, 16, KA).transpose(1, 2, 3, 0)  # [NT,16,KA,B]
        ii = np.arange(16)
        xb[:, ii, :, :, ii] = xsrc.transpose(1, 0, 2, 3)
        xb = np.ascontiguousarray(xb.reshape(NT, 128, B * 16))
        maps.append({
            "xb": xb, "w16": w16, "w2": w2, "xt16": xt16, "xt2k": xt2k,
            "b": b,
        })
    return maps


def kernel(x, W, b):
    runner = get_runner()
    res = runner(make_in_maps(x, W, b))
    return np.asarray(res[0]["y"], dtype=np.float32)
